# revision 9
# baseline (speedup 1.0000x reference)
"""Trainium2 Bass kernel for nn_CPRLinearFused (quantized linear).

Computes out = x @ dequant(weight_int8, scales) + bias where weights are
int8 with per-group (group=128 along K) per-output-channel scales.

Strategy:
  - Host: dequantize W to bf16 (exact int8 values * fp32 scales, rounded
    to bf16), transpose x to xT [K, M] bf16.
  - Device (8 NeuronCores, column-parallel over N): each core runs a
    bf16 GEMM  out_slice[M, N/8] = xT.T @ W_slice  accumulated in fp32
    PSUM, via the tuned matmul_tile_kernel.
  - Host: gather column slices, add bias in fp32.
"""

import numpy as np
import ml_dtypes

import concourse.bass as bass
import concourse.mybir as mybir
import concourse.tile as tile
from concourse.bass_utils import run_bass_kernel_spmd
from concourse.kernels.tile_matmul import matmul_tile_kernel

B, S, K, N = 8, 64, 8192, 16384
M = B * S  # 512
GROUP = 128
G = K // GROUP  # 64
NCORES = 8
NSH = N // NCORES  # 2048 output columns per core

_NC = None
LAST_RESULTS = None  # BassKernelResults of the most recent run (for profiling)
LAST_IN_MAPS = None  # per-core input maps of the most recent run (for benching)


_MAX_SYNC_WAITS = 4  # this walrus build rejects >4 sync waits per instruction
_MAX_SYNC_WAITS_DMA = 1  # and >1 on DMA pseudo-instructions


def _split_sync_waits(nc):
    """Split instructions carrying more than max_waits sem waits.

    The neuronxcc walrus in this container errors with "Too many sync wait
    commands" when one instruction waits on >4 semaphores (Tile's terminal
    drain waits on ~11).  Waiting is sequential per engine sequencer, so
    hoisting the excess waits onto no-ops directly before the instruction is
    semantically identical.
    """
    counter = [0]
    for b in nc.m.functions[0].blocks:
        new_insts = []
        for inst in b.instructions:
            max_waits = _MAX_SYNC_WAITS_DMA  # 1 everywhere: engine limits vary
            si = inst.sync_info
            if si is not None and si.on_wait and len(si.on_wait) > max_waits:
                waits = list(si.on_wait)
                chunks = [
                    waits[i : i + max_waits] for i in range(0, len(waits), max_waits)
                ]
                for chunk in chunks[:-1]:
                    counter[0] += 1
                    nop = mybir.InstNoOp(
                        name=f"split_wait_nop_{counter[0]}",
                        engine=inst.engine,
                        sync_info=mybir.SyncInfo(on_wait=chunk, on_update=[]),
                    )
                    new_insts.append(nop)
                si.on_wait = chunks[-1]
            new_insts.append(inst)
        b.instructions[:] = new_insts


def _build():
    global _NC
    if _NC is None:
        nc = bass.Bass()
        xT = nc.declare_dram_parameter("xT", [K, M], mybir.dt.bfloat16, isOutput=False)
        w = nc.declare_dram_parameter("w", [K, NSH], mybir.dt.bfloat16, isOutput=False)
        out = nc.declare_dram_parameter("out", [M, NSH], mybir.dt.float32, isOutput=True)
        with tile.TileContext(nc) as tc:
            matmul_tile_kernel(
                tc,
                xT[:],
                w[:],
                out[:],
                cache_tiles=True,
            )
        _split_sync_waits(nc)
        _NC = nc
    return _NC


def kernel(x, weight_int8, scales, bias):
    global LAST_RESULTS
    x = np.asarray(x, dtype=np.float32)
    weight_int8 = np.asarray(weight_int8)
    scales = np.asarray(scales, dtype=np.float32)
    bias = np.asarray(bias, dtype=np.float32)

    bf16 = ml_dtypes.bfloat16
    wdq = (
        (weight_int8.reshape(G, GROUP, N).astype(np.float32) * scales[:, None, :])
        .reshape(K, N)
        .astype(bf16)
    )
    xT = np.ascontiguousarray(x.reshape(M, K).astype(bf16).T)

    in_maps = [
        {"xT": xT, "w": np.ascontiguousarray(wdq[:, i * NSH : (i + 1) * NSH])}
        for i in range(NCORES)
    ]
    nc = _build()
    global LAST_IN_MAPS
    LAST_IN_MAPS = in_maps
    res = run_bass_kernel_spmd(nc, in_maps, list(range(NCORES)))
    LAST_RESULTS = res
    out = np.concatenate(
        [res.results[i]["out"] for i in range(NCORES)], axis=1
    ).astype(np.float32)
    out = out + bias[None, :]
    return out.reshape(B, S, N)


# revision 12
# speedup vs baseline: 611.6179x; 611.6179x over previous
"""Trainium2 Bass kernel for nn_CPRLinearFused (quantized linear).

Computes out = x @ dequant(weight_int8, scales) + bias where weights are
int8 with per-group (group=128 along K) per-output-channel scales.

Strategy:
  - Host: dequantize W to bf16 (exact int8 values * fp32 scales, rounded
    to bf16), transpose x to xT [K, M] bf16.
  - Device (8 NeuronCores, column-parallel over N): each core runs a
    bf16 GEMM  out_slice[M, N/8] = xT.T @ W_slice  accumulated in fp32
    PSUM, via the tuned matmul_tile_kernel.
  - Host: gather column slices, add bias in fp32.
"""

import numpy as np
import ml_dtypes

import concourse.bass as bass
import concourse.mybir as mybir
import concourse.tile as tile
from concourse.bass_utils import run_bass_kernel_spmd
from concourse.kernels.tile_matmul import matmul_tile_kernel

B, S, K, N = 8, 64, 8192, 16384
M = B * S  # 512
GROUP = 128
G = K // GROUP  # 64
NCORES = 8
NSH = N // NCORES  # 2048 output columns per core

_NC = None
LAST_RESULTS = None  # BassKernelResults of the most recent run (for profiling)
LAST_IN_MAPS = None  # per-core input maps of the most recent run (for benching)


_MAX_SYNC_WAITS = 4  # this walrus build rejects >4 sync waits per instruction
_MAX_SYNC_WAITS_DMA = 1  # and >1 on DMA pseudo-instructions


def _split_sync_waits(nc):
    """Split instructions carrying more than max_waits sem waits.

    The neuronxcc walrus in this container errors with "Too many sync wait
    commands" when one instruction waits on >4 semaphores (Tile's terminal
    drain waits on ~11).  Waiting is sequential per engine sequencer, so
    hoisting the excess waits onto no-ops directly before the instruction is
    semantically identical.
    """
    counter = [0]
    for b in nc.m.functions[0].blocks:
        new_insts = []
        for inst in b.instructions:
            max_waits = _MAX_SYNC_WAITS_DMA  # 1 everywhere: engine limits vary
            si = inst.sync_info
            if si is not None and si.on_wait and len(si.on_wait) > max_waits:
                waits = list(si.on_wait)
                chunks = [
                    waits[i : i + max_waits] for i in range(0, len(waits), max_waits)
                ]
                for chunk in chunks[:-1]:
                    counter[0] += 1
                    nop = mybir.InstNoOp(
                        name=f"split_wait_nop_{counter[0]}",
                        engine=inst.engine,
                        sync_info=mybir.SyncInfo(on_wait=chunk, on_update=[]),
                    )
                    new_insts.append(nop)
                si.on_wait = chunks[-1]
            new_insts.append(inst)
        b.instructions[:] = new_insts


def _build(repeats=1):
    """Build the per-core Bass program. repeats>1 replicates the GEMM body
    inside one NEFF (used only for differential timing in test harnesses)."""
    global _NC
    if repeats == 1 and _NC is not None:
        return _NC
    nc = bass.Bass()
    xT = nc.declare_dram_parameter("xT", [K, M], mybir.dt.bfloat16, isOutput=False)
    w = nc.declare_dram_parameter("w", [K, NSH], mybir.dt.bfloat16, isOutput=False)
    out = nc.declare_dram_parameter("out", [M, NSH], mybir.dt.float32, isOutput=True)
    with tile.TileContext(nc) as tc:
        for _ in range(repeats):
            matmul_tile_kernel(
                tc,
                xT[:],
                w[:],
                out[:],
                cache_tiles=True,
            )
    _split_sync_waits(nc)
    if repeats == 1:
        _NC = nc
    return nc


def _run_spmd(nc, in_maps):
    """run_bass_kernel_spmd with two defensive fallbacks:
    - if the axon NTFF trace hook is missing (BASS_TRACE set but
      antenv.axon_hooks not shipped in this container), disable tracing;
    - retry once on transient device errors (mesh desync / unrecoverable).
    """
    import os

    core_ids = list(range(NCORES))
    try:
        return run_bass_kernel_spmd(nc, in_maps, core_ids)
    except (ModuleNotFoundError, ImportError):
        os.environ["BASS_NEVER_TRACE"] = "1"
        return run_bass_kernel_spmd(nc, in_maps, core_ids)
    except Exception as e:  # transient NRT/axon failures
        msg = str(e)
        if "UNRECOVERABLE" in msg or "desynced" in msg or "UNAVAILABLE" in msg:
            return run_bass_kernel_spmd(nc, in_maps, core_ids)
        raise


def kernel(x, weight_int8, scales, bias):
    global LAST_RESULTS
    x = np.asarray(x, dtype=np.float32)
    weight_int8 = np.asarray(weight_int8)
    scales = np.asarray(scales, dtype=np.float32)
    bias = np.asarray(bias, dtype=np.float32)

    bf16 = ml_dtypes.bfloat16
    wdq = (
        (weight_int8.reshape(G, GROUP, N).astype(np.float32) * scales[:, None, :])
        .reshape(K, N)
        .astype(bf16)
    )
    xT = np.ascontiguousarray(x.reshape(M, K).astype(bf16).T)

    in_maps = [
        {"xT": xT, "w": np.ascontiguousarray(wdq[:, i * NSH : (i + 1) * NSH])}
        for i in range(NCORES)
    ]
    nc = _build()
    global LAST_IN_MAPS
    LAST_IN_MAPS = in_maps
    res = _run_spmd(nc, in_maps)
    LAST_RESULTS = res
    out = np.concatenate(
        [res.results[i]["out"] for i in range(NCORES)], axis=1
    ).astype(np.float32)
    out = out + bias[None, :]
    return out.reshape(B, S, N)
